# revision 2
# baseline (speedup 1.0000x reference)
"""Fused RNN cell on 8 Trainium2 NeuronCores.

Reference computation (fp32):
    combined   = [x, hidden]                      [B=4096, I+H=4096]
    new_hidden = tanh(combined @ W_ih^T + b_ih)   [B, H=2048]
    output     = new_hidden @ W_ho^T + b_ho       [B, O=2048]
    returns (output, new_hidden)

Strategy: data-parallel over the batch — each of the 8 cores processes 512
batch rows with replicated weights; no collectives. All operand layout
transforms (transposes into PE-friendly [K-partition, free] form) happen on
the host so every device DMA is a fat, fully contiguous transfer:

    c   [128, 32, 512]      cL[ki, ko, b]       = combined[b, ko*128+ki]
    w1  [128, 32, 16, 128]  w1L[ki, ko, hc, h]  = W_ih[hc*128+h, ko*128+ki]
    w2  [128, 16, 16, 128]  w2L[hi, ho, oc, o]  = W_ho[oc*128+o, ho*128+hi]
    b1  [128, 16]           b1L[p, hc]          = b_ih[hc*128+p]

Matmuls run in bf16 (same full-rate PE throughput as fp32r — 1 column/cycle
— but half the HBM traffic, and FWL halves the LDWEIGHTS cost; measured rms
rel err ~4e-3 vs the 2e-2 gate). PSUM accumulation stays fp32. mm1 produces
nh^T [h, b] bf16 tiles in SBUF, which feed mm2 directly as the streaming
operand; mm2 produces out^T [o, b] stored bf16. Outputs are un-transposed
and upcast on the host after the gather; b_ho is added on the host.

Loop structure: h-chunks (and o-chunks) are processed in groups of 8, one
PSUM bank per chunk. Each inner step streams a 512 KiB two-ko weight slice
[128, 2, 8, 128] (plus, in the first group, two [128, 512] c-chunks) and
issues 16 accumulating matmuls, so DMA stays deep and fat while the PE runs
back-to-back bf16 matmuls (~216 ns/MM). Stores ride both HWDGE rings,
deferred one group so a store waiting on compute never head-of-line blocks
the load ring. Dummy matmuls at t=0 warm the PE clock gate (HAM) and
preload the ACT tanh table during the initial DMA ramp.
"""

import numpy as np
import ml_dtypes

import concourse.bass as bass
import concourse.mybir as mybir
import concourse.tile as tile
from concourse import bacc, bass_utils

NCORES = 8
B, I, H, O = 4096, 2048, 2048, 2048
BC = B // NCORES          # 512 batch rows per core
K1 = I + H                # mm1 contraction dim, 4096
KO1 = K1 // 128           # 32 k-chunks for mm1
HC = H // 128             # 16 h-chunks
OC = O // 128             # 16 o-chunks
G = 8                     # h/o-chunks per PSUM group (8 banks)
P = 128
F32 = mybir.dt.float32
BF16 = mybir.dt.bfloat16
AF = mybir.ActivationFunctionType
NPBF16 = ml_dtypes.bfloat16


def _build():
    nc = bacc.Bacc("TRN2", target_bir_lowering=False)

    c = nc.dram_tensor("c", [P, KO1, BC], BF16, kind="ExternalInput")
    w1 = nc.dram_tensor("w1", [P, KO1, HC, P], BF16, kind="ExternalInput")
    b1 = nc.dram_tensor("b1", [P, HC], F32, kind="ExternalInput")
    w2 = nc.dram_tensor("w2", [P, HC, OC, P], BF16, kind="ExternalInput")
    nhT = nc.dram_tensor("nhT", [H, BC], BF16, kind="ExternalOutput")
    outT = nc.dram_tensor("outT", [O, BC], BF16, kind="ExternalOutput")

    with tile.TileContext(nc) as tc:
        with tc.tile_pool(name="cpool", bufs=1) as cpool, \
             tc.tile_pool(name="wpool", bufs=10) as wpool, \
             tc.tile_pool(name="nhpool", bufs=1) as nhpool, \
             tc.tile_pool(name="opool", bufs=8) as opool, \
             tc.tile_pool(name="bpool", bufs=1) as bpool, \
             tc.tile_pool(name="ps", bufs=8, space="PSUM") as ps:

            b1_sb = bpool.tile([P, HC], F32)
            # b_ih isn't needed until the first group drains; keep it off
            # the sync HWDGE ring entirely (SWDGE via GpSimd). b_ho is
            # added on the host after the gather.
            nc.gpsimd.dma_start(b1_sb[:], b1[:])

            c_sb = cpool.tile([P, KO1, BC], BF16)
            nh_sb = nhpool.tile([P, HC, BC], BF16)

            # PE warm-up: the HAM clock gate holds the PE at 1.2 GHz until
            # it has been busy ~3.4 µs. Dummy matmuls (no data deps beyond
            # one memset) keep the PE active while the first input tiles
            # stream in, so real matmuls start at 2.4 GHz.
            warm_sb = bpool.tile([P, P], BF16)
            nc.vector.memset(warm_sb[:], 0.0)
            # Preload the ACT tanh table set (~2.7 us) during the DMA-bound
            # ramp instead of at the first group's drain.
            act_warm = bpool.tile([1, 1], F32)
            nc.scalar.activation(act_warm[:], warm_sb[:1, :1], AF.Tanh)

            # Stores are deferred one group: group g's stores are emitted
            # after group g+1's loads, so when the sync sequencer reaches
            # them the producing compute finished long ago and the ring
            # never head-of-line blocks on a store waiting for compute.
            deferred = []

            def flush_deferred():
                for fn in deferred:
                    fn()
                deferred.clear()

            # mm1: nh^T[h, b] = tanh(W_ih @ combined^T + b_ih)
            # G-sized PSUM groups ping-pong across the 8 banks: while one
            # group's banks drain through ACT, the next group accumulates
            # — group boundaries cost the PE almost nothing.
            for g in range(HC // G):
                psums = [ps.tile([P, BC], F32, tag="ps", name=f"ps{i}")
                         for i in range(G)]
                if g == 0:
                    # ~40 cold matmuls at ~107 ns each cover the ~3.4 us
                    # HAM window plus the first tiles' DMA ramp.
                    for _ in range(40):
                        nc.tensor.matmul(
                            psums[G - 1][:, :P], lhsT=warm_sb[:],
                            rhs=warm_sb[:],
                            start=True, stop=True, skip_group_check=True,
                        )
                for ko0 in range(0, KO1, 2):
                    if g == 0:
                        nc.sync.dma_start(c_sb[:, ko0:ko0 + 2], c[:, ko0:ko0 + 2])
                    w1_sb = wpool.tile([P, 2, G, P], BF16, tag="w")
                    nc.sync.dma_start(
                        w1_sb[:], w1[:, ko0:ko0 + 2, g * G:(g + 1) * G])
                    for kk in range(2):
                        for i in range(G):
                            nc.tensor.matmul(
                                psums[i][:],
                                lhsT=w1_sb[:, kk, i],
                                rhs=c_sb[:, ko0 + kk],
                                start=(ko0 + kk == 0),
                                stop=(ko0 + kk == KO1 - 1),
                            )
                flush_deferred()
                for i in range(G):
                    hc = g * G + i
                    nc.scalar.activation(
                        nh_sb[:, hc], psums[i][:], AF.Tanh,
                        bias=b1_sb[:, hc:hc + 1],
                    )
                    deferred.append(
                        lambda hc=hc: nc.sync.dma_start(
                            nhT[hc * P:(hc + 1) * P, :], nh_sb[:, hc])
                    )

            # mm2: out^T[o, b] = W_ho @ nh^T + b_ho
            # Groups of [8, 4, 4] o-chunks: the two trailing 4-groups
            # ping-pong through the 8 PSUM banks (no boundary stall) and
            # the final drain chain is half as long, shortening the tail.
            for g0, gsz in ((0, 8), (8, 4), (12, 4)):
                psums = [ps.tile([P, BC], F32, tag="ps", name=f"ps{i}")
                         for i in range(gsz)]
                for ho0 in range(0, HC, 2):
                    w2_sb = wpool.tile([P, 2, G, P], BF16, tag="w", name="w2_sb")[:, :, :gsz]
                    nc.sync.dma_start(
                        w2_sb[:], w2[:, ho0:ho0 + 2, g0:g0 + gsz])
                    for kk in range(2):
                        for i in range(gsz):
                            nc.tensor.matmul(
                                psums[i][:],
                                lhsT=w2_sb[:, kk, i],
                                rhs=nh_sb[:, ho0 + kk],
                                start=(ho0 + kk == 0),
                                stop=(ho0 + kk == HC - 1),
                            )
                flush_deferred()
                # Evict PSUM through both DVE and ACT in parallel (raw
                # copies; b_ho is added on the host). ACT-evicted tiles
                # store via the ACT HWDGE ring right behind their copy;
                # DVE-evicted tiles store via the sync ring, deferred one
                # group so the ring never waits on the copy.
                for i in range(gsz):
                    oc = g0 + i
                    o_sb = opool.tile([P, BC], BF16, tag="osb")
                    if i % 2:
                        nc.scalar.activation(o_sb[:], psums[i][:], AF.Copy)
                        nc.scalar.dma_start(
                            outT[oc * P:(oc + 1) * P, :], o_sb[:])
                    else:
                        nc.vector.tensor_copy(o_sb[:], psums[i][:])
                        deferred.append(
                            lambda oc=oc, o_sb=o_sb: nc.sync.dma_start(
                                outT[oc * P:(oc + 1) * P, :], o_sb[:])
                        )
            flush_deferred()

    nc.compile()
    return nc


def _shard_inputs(x, hidden, W_ih, b_ih, W_ho, b_ho):
    combined = np.concatenate([x, hidden], axis=1)  # [B, K1]
    w1L = np.ascontiguousarray(
        W_ih.reshape(HC, P, KO1, P).transpose(3, 2, 0, 1)
    ).astype(NPBF16)  # [ki, ko, hc, h]
    w2L = np.ascontiguousarray(
        W_ho.reshape(OC, P, HC, P).transpose(3, 2, 0, 1)
    ).astype(NPBF16)  # [hi, ho, oc, o]
    b1L = np.ascontiguousarray(b_ih.reshape(HC, P).T)
    in_maps = []
    for cix in range(NCORES):
        cc = combined[cix * BC:(cix + 1) * BC]  # [BC, K1]
        cL = np.ascontiguousarray(
            cc.reshape(BC, KO1, P).transpose(2, 1, 0)).astype(NPBF16)
        in_maps.append(
            {"c": cL, "w1": w1L, "b1": b1L, "w2": w2L}
        )
    return in_maps


def _run(in_maps, **kwargs):
    nc = _build()
    return bass_utils.run_bass_kernel_spmd(
        nc, in_maps, core_ids=list(range(NCORES)), **kwargs
    )


def kernel(x, hidden, W_ih, b_ih, W_ho, b_ho):
    x = np.asarray(x, dtype=np.float32)
    hidden = np.asarray(hidden, dtype=np.float32)
    W_ih = np.asarray(W_ih, dtype=np.float32)
    b_ih = np.asarray(b_ih, dtype=np.float32)
    W_ho = np.asarray(W_ho, dtype=np.float32)
    b_ho = np.asarray(b_ho, dtype=np.float32)

    in_maps = _shard_inputs(x, hidden, W_ih, b_ih, W_ho, b_ho)
    res = _run(in_maps)
    output = np.concatenate(
        [r["outT"].T.astype(np.float32) for r in res.results], axis=0) + b_ho
    new_hidden = np.concatenate(
        [r["nhT"].T.astype(np.float32) for r in res.results], axis=0)
    return output, new_hidden


# revision 9
# speedup vs baseline: 1.1612x; 1.1612x over previous
"""Fused RNN cell on 8 Trainium2 NeuronCores.

Reference computation (fp32):
    combined   = [x, hidden]                      [B=4096, I+H=4096]
    new_hidden = tanh(combined @ W_ih^T + b_ih)   [B, H=2048]
    output     = new_hidden @ W_ho^T + b_ho       [B, O=2048]
    returns (output, new_hidden)

Strategy: data-parallel over the batch — each of the 8 cores processes 512
batch rows with replicated weights; no collectives. All operand layout
transforms (transposes into PE-friendly [K-partition, free] form) happen on
the host so every device DMA is a fat, fully contiguous transfer:

    c   [128, 32, 512]      cL[ki, ko, b]       = combined[b, ko*128+ki]
    w1  [128, 32, 16, 128]  w1L[ki, ko, hc, h]  = W_ih[hc*128+h, ko*128+ki]
    w2  [128, 16, 16, 128]  w2L[hi, ho, oc, o]  = W_ho[oc*128+o, ho*128+hi]
    b1  [128, 16]           b1L[p, hc]          = b_ih[hc*128+p]

Matmuls run in fp16 (full-rate on the PE — measured 216 ns per 512-col MM
in isolation — with half the HBM traffic of fp32, FWL-accelerated weight
loads, and ~3e-4 rounding error vs the 2e-2 gate). PSUM accumulation is
fp32. mm1 produces nh^T [h, b] fp16 tiles in SBUF, which feed mm2 directly
as the streaming operand; mm2 produces out^T [o, b] stored fp16. Outputs
are un-transposed and upcast on the host after the gather; b_ho is added
on the host.

Loop structure: h-chunks (and o-chunks) are processed in groups of 8, one
PSUM bank per chunk. Each inner step streams a 512 KiB two-ko weight slice
[128, 2, 8, 128] (plus, in the first group, two [128, 512] c-chunks) and
issues 16 accumulating matmuls, so DMA stays deep and fat while the PE runs
back-to-back bf16 matmuls (~216 ns/MM). Stores ride both HWDGE rings,
deferred one group so a store waiting on compute never head-of-line blocks
the load ring. Dummy matmuls at t=0 warm the PE clock gate (HAM) and
preload the ACT tanh table during the initial DMA ramp.
"""

import numpy as np
import ml_dtypes

import concourse.bass as bass
import concourse.mybir as mybir
import concourse.tile as tile
from concourse import bacc, bass_utils

NCORES = 8
B, I, H, O = 4096, 2048, 2048, 2048
BC = B // NCORES          # 512 batch rows per core
K1 = I + H                # mm1 contraction dim, 4096
KO1 = K1 // 128           # 32 k-chunks for mm1
HC = H // 128             # 16 h-chunks
OC = O // 128             # 16 o-chunks
G = 8                     # h/o-chunks per PSUM group (8 banks)
P = 128
F32 = mybir.dt.float32
F16 = mybir.dt.float16
AF = mybir.ActivationFunctionType
NPF16 = np.float16


def _build():
    nc = bacc.Bacc("TRN2", target_bir_lowering=False)

    c = nc.dram_tensor("c", [P, KO1, BC], F16, kind="ExternalInput")
    w1 = nc.dram_tensor("w1", [P, KO1, HC, P], F16, kind="ExternalInput")
    b1 = nc.dram_tensor("b1", [P, HC], F32, kind="ExternalInput")
    w2 = nc.dram_tensor("w2", [P, HC, OC, P], F16, kind="ExternalInput")
    nhT = nc.dram_tensor("nhT", [H, BC], F16, kind="ExternalOutput")
    outT = nc.dram_tensor("outT", [O, BC], F16, kind="ExternalOutput")

    with tile.TileContext(nc) as tc:
        with tc.tile_pool(name="cpool", bufs=1) as cpool, \
             tc.tile_pool(name="wpool", bufs=10) as wpool, \
             tc.tile_pool(name="nhpool", bufs=1) as nhpool, \
             tc.tile_pool(name="opool", bufs=8) as opool, \
             tc.tile_pool(name="bpool", bufs=1) as bpool, \
             tc.tile_pool(name="ps", bufs=8, space="PSUM") as ps:

            b1_sb = bpool.tile([P, HC], F32)
            # b_ih isn't needed until the first group drains; keep it off
            # the sync HWDGE ring entirely (SWDGE via GpSimd). b_ho is
            # added on the host after the gather.
            nc.gpsimd.dma_start(b1_sb[:], b1[:])

            c_sb = cpool.tile([P, KO1, BC], F16)
            nh_sb = nhpool.tile([P, HC, BC], F16)

            # PE warm-up: the HAM clock gate holds the PE at 1.2 GHz until
            # it has been busy ~3.4 µs. Dummy matmuls (no data deps beyond
            # one memset) keep the PE active while the first input tiles
            # stream in, so real matmuls start at 2.4 GHz.
            warm_sb = bpool.tile([P, P], F16)
            nc.vector.memset(warm_sb[:], 0.0)
            # Preload the ACT tanh table set (~2.7 us) during the DMA-bound
            # ramp instead of at the first group's drain.
            act_warm = bpool.tile([1, 1], F32)
            nc.scalar.activation(act_warm[:], warm_sb[:1, :1], AF.Tanh)

            # Stores are deferred one group: group g's stores are emitted
            # after group g+1's loads, so when the sync sequencer reaches
            # them the producing compute finished long ago and the ring
            # never head-of-line blocks on a store waiting for compute.
            deferred = []

            def flush_deferred():
                for fn in deferred:
                    fn()
                deferred.clear()

            # mm1: nh^T[h, b] = tanh(W_ih @ combined^T + b_ih)
            # G-sized PSUM groups ping-pong across the 8 banks: while one
            # group's banks drain through ACT, the next group accumulates
            # — group boundaries cost the PE almost nothing.
            for g in range(HC // G):
                psums = [ps.tile([P, BC], F32, tag="ps", name=f"ps{i}")
                         for i in range(G)]
                if g == 0:
                    # ~40 cold matmuls at ~107 ns each cover the ~3.4 us
                    # HAM window plus the first tiles' DMA ramp.
                    for _ in range(40):
                        nc.tensor.matmul(
                            psums[G - 1][:, :P], lhsT=warm_sb[:],
                            rhs=warm_sb[:],
                            start=True, stop=True, skip_group_check=True,
                        )
                for ko0 in range(0, KO1, 2):
                    if g == 0:
                        nc.sync.dma_start(c_sb[:, ko0:ko0 + 2], c[:, ko0:ko0 + 2])
                    w1_sb = wpool.tile([P, 2, G, P], F16, tag="w")
                    nc.sync.dma_start(
                        w1_sb[:], w1[:, ko0:ko0 + 2, g * G:(g + 1) * G])
                    for kk in range(2):
                        for i in range(G):
                            nc.tensor.matmul(
                                psums[i][:],
                                lhsT=w1_sb[:, kk, i],
                                rhs=c_sb[:, ko0 + kk],
                                start=(ko0 + kk == 0),
                                stop=(ko0 + kk == KO1 - 1),
                            )
                flush_deferred()
                for i in range(G):
                    hc = g * G + i
                    nc.scalar.activation(
                        nh_sb[:, hc], psums[i][:], AF.Tanh,
                        bias=b1_sb[:, hc:hc + 1],
                    )
                    deferred.append(
                        lambda hc=hc: nc.sync.dma_start(
                            nhT[hc * P:(hc + 1) * P, :], nh_sb[:, hc])
                    )

            # mm2: out^T[o, b] = W_ho @ nh^T + b_ho
            # Groups of [8, 4, 4] o-chunks: the two trailing 4-groups
            # ping-pong through the 8 PSUM banks (no boundary stall) and
            # the final drain chain is half as long, shortening the tail.
            for g0, gsz in ((0, 8), (8, 4), (12, 4)):
                psums = [ps.tile([P, BC], F32, tag="ps", name=f"ps{i}")
                         for i in range(gsz)]
                for ho0 in range(0, HC, 2):
                    w2_sb = wpool.tile([P, 2, G, P], F16, tag="w", name="w2_sb")[:, :, :gsz]
                    nc.sync.dma_start(
                        w2_sb[:], w2[:, ho0:ho0 + 2, g0:g0 + gsz])
                    for kk in range(2):
                        for i in range(gsz):
                            nc.tensor.matmul(
                                psums[i][:],
                                lhsT=w2_sb[:, kk, i],
                                rhs=nh_sb[:, ho0 + kk],
                                start=(ho0 + kk == 0),
                                stop=(ho0 + kk == HC - 1),
                            )
                flush_deferred()
                # Evict PSUM through both DVE and ACT in parallel (raw
                # copies; b_ho is added on the host). ACT-evicted tiles
                # store via the ACT HWDGE ring right behind their copy;
                # DVE-evicted tiles store via the sync ring, deferred one
                # group so the ring never waits on the copy.
                for i in range(gsz):
                    oc = g0 + i
                    o_sb = opool.tile([P, BC], F16, tag="osb")
                    if i % 2:
                        nc.scalar.activation(o_sb[:], psums[i][:], AF.Copy)
                        nc.scalar.dma_start(
                            outT[oc * P:(oc + 1) * P, :], o_sb[:])
                    else:
                        nc.vector.tensor_copy(o_sb[:], psums[i][:])
                        deferred.append(
                            lambda oc=oc, o_sb=o_sb: nc.sync.dma_start(
                                outT[oc * P:(oc + 1) * P, :], o_sb[:])
                        )
            flush_deferred()

    nc.compile()
    return nc


def _shard_inputs(x, hidden, W_ih, b_ih, W_ho, b_ho):
    combined = np.concatenate([x, hidden], axis=1)  # [B, K1]
    w1L = np.ascontiguousarray(
        W_ih.reshape(HC, P, KO1, P).transpose(3, 2, 0, 1)
    ).astype(NPF16)  # [ki, ko, hc, h]
    w2L = np.ascontiguousarray(
        W_ho.reshape(OC, P, HC, P).transpose(3, 2, 0, 1)
    ).astype(NPF16)  # [hi, ho, oc, o]
    b1L = np.ascontiguousarray(b_ih.reshape(HC, P).T)
    in_maps = []
    for cix in range(NCORES):
        cc = combined[cix * BC:(cix + 1) * BC]  # [BC, K1]
        cL = np.ascontiguousarray(
            cc.reshape(BC, KO1, P).transpose(2, 1, 0)).astype(NPF16)
        in_maps.append(
            {"c": cL, "w1": w1L, "b1": b1L, "w2": w2L}
        )
    return in_maps


def _run(in_maps, **kwargs):
    nc = _build()
    return bass_utils.run_bass_kernel_spmd(
        nc, in_maps, core_ids=list(range(NCORES)), **kwargs
    )


def kernel(x, hidden, W_ih, b_ih, W_ho, b_ho):
    x = np.asarray(x, dtype=np.float32)
    hidden = np.asarray(hidden, dtype=np.float32)
    W_ih = np.asarray(W_ih, dtype=np.float32)
    b_ih = np.asarray(b_ih, dtype=np.float32)
    W_ho = np.asarray(W_ho, dtype=np.float32)
    b_ho = np.asarray(b_ho, dtype=np.float32)

    in_maps = _shard_inputs(x, hidden, W_ih, b_ih, W_ho, b_ho)
    res = _run(in_maps)
    output = np.concatenate(
        [r["outT"].T.astype(np.float32) for r in res.results], axis=0) + b_ho
    new_hidden = np.concatenate(
        [r["nhT"].T for r in res.results], axis=0).astype(np.float32)
    return output, new_hidden


# revision 13
# speedup vs baseline: 1.1916x; 1.0262x over previous
"""Fused RNN cell on 8 Trainium2 NeuronCores.

Reference computation (fp32):
    combined   = [x, hidden]                      [B=4096, I+H=4096]
    new_hidden = tanh(combined @ W_ih^T + b_ih)   [B, H=2048]
    output     = new_hidden @ W_ho^T + b_ho       [B, O=2048]
    returns (output, new_hidden)

Strategy: data-parallel over the batch — each of the 8 cores processes 512
batch rows with replicated weights; no collectives. All operand layout
transforms (transposes into PE-friendly [K-partition, free] form) happen on
the host so every device DMA is a fat, fully contiguous transfer:

    c   [128, 32, 512]      cL[ki, ko, b]       = combined[b, ko*128+ki]
    w1  [128, 32, 16, 128]  w1L[ki, ko, hc, h]  = W_ih[hc*128+h, ko*128+ki]
    w2  [128, 16, 16, 128]  w2L[hi, ho, oc, o]  = W_ho[oc*128+o, ho*128+hi]
    b1  [128, 16]           b1L[p, hc]          = b_ih[hc*128+p]

Matmuls run in fp16 (full-rate on the PE — measured 216 ns per 512-col MM
in isolation — with half the HBM traffic of fp32, FWL-accelerated weight
loads, and ~3e-4 rounding error vs the 2e-2 gate). PSUM accumulation is
fp32. mm1 produces nh^T [h, b] fp16 tiles in SBUF, which feed mm2 directly
as the streaming operand; mm2 produces out^T [o, b] stored fp16. Outputs
are un-transposed and upcast on the host after the gather; b_ho is added
on the host.

Loop structure: h-chunks (and o-chunks) are processed in groups of 8, one
PSUM bank per chunk. Each inner step streams a 512 KiB two-ko weight slice
[128, 2, 8, 128] (plus, in the first group, two [128, 512] c-chunks) and
issues 16 accumulating matmuls, so DMA stays deep and fat while the PE runs
back-to-back bf16 matmuls (~216 ns/MM). Stores ride both HWDGE rings,
deferred one group so a store waiting on compute never head-of-line blocks
the load ring. Dummy matmuls at t=0 warm the PE clock gate (HAM) and
preload the ACT tanh table during the initial DMA ramp.
"""

import numpy as np
import ml_dtypes

import concourse.bass as bass
import concourse.mybir as mybir
import concourse.tile as tile
from concourse import bacc, bass_utils

NCORES = 8
B, I, H, O = 4096, 2048, 2048, 2048
BC = B // NCORES          # 512 batch rows per core
K1 = I + H                # mm1 contraction dim, 4096
KO1 = K1 // 128           # 32 k-chunks for mm1
HC = H // 128             # 16 h-chunks
OC = O // 128             # 16 o-chunks
G = 8                     # h/o-chunks per PSUM group (8 banks)
P = 128
F32 = mybir.dt.float32
F16 = mybir.dt.float16
AF = mybir.ActivationFunctionType
NPF16 = np.float16


def _build():
    nc = bacc.Bacc("TRN2", target_bir_lowering=False)

    c = nc.dram_tensor("c", [P, KO1, BC], F16, kind="ExternalInput")
    w1 = nc.dram_tensor("w1", [P, KO1, HC, P], F16, kind="ExternalInput")
    b1 = nc.dram_tensor("b1", [P, HC], F32, kind="ExternalInput")
    w2 = nc.dram_tensor("w2", [P, HC, OC, P], F16, kind="ExternalInput")
    nhT = nc.dram_tensor("nhT", [H, BC], F16, kind="ExternalOutput")
    outT = nc.dram_tensor("outT", [O, BC], F16, kind="ExternalOutput")

    with tile.TileContext(nc) as tc:
        with tc.tile_pool(name="cpool", bufs=1) as cpool, \
             tc.tile_pool(name="wpool", bufs=10) as wpool, \
             tc.tile_pool(name="nhpool", bufs=1) as nhpool, \
             tc.tile_pool(name="opool", bufs=8) as opool, \
             tc.tile_pool(name="bpool", bufs=1) as bpool, \
             tc.tile_pool(name="ps", bufs=8, space="PSUM") as ps:

            b1_sb = bpool.tile([P, HC], F32)
            # b_ih isn't needed until the first group drains; keep it off
            # the sync HWDGE ring entirely (SWDGE via GpSimd). b_ho is
            # added on the host after the gather.
            nc.gpsimd.dma_start(b1_sb[:], b1[:])

            c_sb = cpool.tile([P, KO1, BC], F16)
            nh_sb = nhpool.tile([P, HC, BC], F16)

            # PE warm-up: the HAM clock gate holds the PE at 1.2 GHz until
            # it has been busy ~3.4 µs. Dummy matmuls (no data deps beyond
            # one memset) keep the PE active while the first input tiles
            # stream in, so real matmuls start at 2.4 GHz. The memset rides
            # GpSimd, whose queue is free right after the preamble (the
            # Vector queue picks up its first instruction ~2 µs later).
            warm_sb = bpool.tile([P, P], F16)
            nc.gpsimd.memset(warm_sb[:], 0.0)

            # Stores are deferred one group: group g's stores are emitted
            # after group g+1's loads, so when the sync sequencer reaches
            # them the producing compute finished long ago and the ring
            # never head-of-line blocks on a store waiting for compute.
            deferred = []

            def flush_deferred():
                for fn in deferred:
                    fn()
                deferred.clear()

            # mm1: nh^T[h, b] = tanh(W_ih @ combined^T + b_ih)
            # G-sized PSUM groups ping-pong across the 8 banks: while one
            # group's banks drain through ACT, the next group accumulates
            # — group boundaries cost the PE almost nothing.
            for g in range(HC // G):
                psums = [ps.tile([P, BC], F32, tag="ps", name=f"ps{i}")
                         for i in range(G)]
                if g == 0:
                    # ~28 cold matmuls at ~107 ns each cover the ~3.4 us
                    # HAM window plus the first tiles' DMA ramp.
                    for _ in range(28):
                        nc.tensor.matmul(
                            psums[G - 1][:, :P], lhsT=warm_sb[:],
                            rhs=warm_sb[:],
                            start=True, stop=True, skip_group_check=True,
                        )
                for ko0 in range(0, KO1, 2):
                    if g == 0:
                        # c rides the ACT HWDGE ring: descriptor pushes for
                        # the first c and w1 tiles then run in parallel on
                        # two queues (~0.7 us each), and during all of
                        # group 0 the sync ring carries only weights.
                        nc.scalar.dma_start(c_sb[:, ko0:ko0 + 2], c[:, ko0:ko0 + 2])
                        if ko0 == 2:
                            # Preload the ACT tanh table set (~1.3 us)
                            # during the ramp, not at the first drain.
                            act_warm = bpool.tile([1, 1], F32)
                            nc.scalar.activation(
                                act_warm[:], warm_sb[:1, :1], AF.Tanh)
                    w1_sb = wpool.tile([P, 2, G, P], F16, tag="w")
                    nc.sync.dma_start(
                        w1_sb[:], w1[:, ko0:ko0 + 2, g * G:(g + 1) * G])
                    for kk in range(2):
                        for i in range(G):
                            nc.tensor.matmul(
                                psums[i][:],
                                lhsT=w1_sb[:, kk, i],
                                rhs=c_sb[:, ko0 + kk],
                                start=(ko0 + kk == 0),
                                stop=(ko0 + kk == KO1 - 1),
                            )
                flush_deferred()
                for i in range(G):
                    hc = g * G + i
                    nc.scalar.activation(
                        nh_sb[:, hc], psums[i][:], AF.Tanh,
                        bias=b1_sb[:, hc:hc + 1],
                    )
                    deferred.append(
                        lambda hc=hc: nc.sync.dma_start(
                            nhT[hc * P:(hc + 1) * P, :], nh_sb[:, hc])
                    )

            # mm2: out^T[o, b] = W_ho @ nh^T + b_ho
            # Groups of [8, 4, 2, 2] o-chunks: trailing groups ping-pong
            # through the 8 PSUM banks (no boundary stall) and shrink so
            # the post-last-matmul drain chain is as short as possible.
            for g0, gsz in ((0, 8), (8, 4), (12, 2), (14, 2)):
                psums = [ps.tile([P, BC], F32, tag="ps", name=f"ps{i}")
                         for i in range(gsz)]
                for ho0 in range(0, HC, 2):
                    w2_sb = wpool.tile([P, 2, G, P], F16, tag="w", name="w2_sb")[:, :, :gsz]
                    nc.sync.dma_start(
                        w2_sb[:], w2[:, ho0:ho0 + 2, g0:g0 + gsz])
                    for kk in range(2):
                        for i in range(gsz):
                            nc.tensor.matmul(
                                psums[i][:],
                                lhsT=w2_sb[:, kk, i],
                                rhs=nh_sb[:, ho0 + kk],
                                start=(ho0 + kk == 0),
                                stop=(ho0 + kk == HC - 1),
                            )
                flush_deferred()
                # Evict PSUM through both DVE and ACT in parallel (raw
                # copies; b_ho is added on the host). ACT-evicted tiles
                # store via the ACT HWDGE ring right behind their copy;
                # DVE-evicted tiles store via the sync ring, deferred one
                # group so the ring never waits on the copy.
                last = (g0 + gsz == OC)
                for i in range(gsz):
                    oc = g0 + i
                    o_sb = opool.tile([P, BC], F16, tag="osb")
                    if i % 2:
                        nc.scalar.activation(o_sb[:], psums[i][:], AF.Copy)
                        nc.scalar.dma_start(
                            outT[oc * P:(oc + 1) * P, :], o_sb[:])
                    else:
                        nc.vector.tensor_copy(o_sb[:], psums[i][:])
                        st = (lambda oc=oc, o_sb=o_sb: nc.sync.dma_start(
                            outT[oc * P:(oc + 1) * P, :], o_sb[:]))
                        if last:
                            st()      # no deferral on the final group
                        else:
                            deferred.append(st)
            flush_deferred()

    nc.compile()
    return nc


def _shard_inputs(x, hidden, W_ih, b_ih, W_ho, b_ho):
    combined = np.concatenate([x, hidden], axis=1)  # [B, K1]
    w1L = np.ascontiguousarray(
        W_ih.reshape(HC, P, KO1, P).transpose(3, 2, 0, 1)
    ).astype(NPF16)  # [ki, ko, hc, h]
    w2L = np.ascontiguousarray(
        W_ho.reshape(OC, P, HC, P).transpose(3, 2, 0, 1)
    ).astype(NPF16)  # [hi, ho, oc, o]
    b1L = np.ascontiguousarray(b_ih.reshape(HC, P).T)
    in_maps = []
    for cix in range(NCORES):
        cc = combined[cix * BC:(cix + 1) * BC]  # [BC, K1]
        cL = np.ascontiguousarray(
            cc.reshape(BC, KO1, P).transpose(2, 1, 0)).astype(NPF16)
        in_maps.append(
            {"c": cL, "w1": w1L, "b1": b1L, "w2": w2L}
        )
    return in_maps


def _run(in_maps, **kwargs):
    nc = _build()
    return bass_utils.run_bass_kernel_spmd(
        nc, in_maps, core_ids=list(range(NCORES)), **kwargs
    )


def kernel(x, hidden, W_ih, b_ih, W_ho, b_ho):
    x = np.asarray(x, dtype=np.float32)
    hidden = np.asarray(hidden, dtype=np.float32)
    W_ih = np.asarray(W_ih, dtype=np.float32)
    b_ih = np.asarray(b_ih, dtype=np.float32)
    W_ho = np.asarray(W_ho, dtype=np.float32)
    b_ho = np.asarray(b_ho, dtype=np.float32)

    in_maps = _shard_inputs(x, hidden, W_ih, b_ih, W_ho, b_ho)
    res = _run(in_maps)
    output = np.concatenate(
        [r["outT"].T.astype(np.float32) for r in res.results], axis=0) + b_ho
    new_hidden = np.concatenate(
        [r["nhT"].T for r in res.results], axis=0).astype(np.float32)
    return output, new_hidden
